# revision 1
# baseline (speedup 1.0000x reference)
"""Modulated deformable conv2d (DCNv2) on Trainium2, data-parallel over batch on 8 NeuronCores.

Per-core pipeline (one batch element per core):
  1. prep:  x [256,3136] f32 -> bf16 -> PE-transpose -> xT [3136,256] bf16 in DRAM
            weight [256,2304] f32 -> bf16 -> PE-transpose -> W' tiles [128(k),256(o)]
  2. coords: offsets+mask -> fractional bilinear weights (position-major, per-partition
            scalars) + int16 gather indices (clipped; invalid taps get weight 0)
  3. gather: dma_gather rows of xT for the 4 bilinear neighbors of all 9 taps
  4. combine: w_lt*v_lt + w_rt*v_rt + w_lb*v_lb + w_rb*v_rb  (1 ACT mul + 3 DVE fused ops)
  5. PE-transpose combined tiles into channel-major rhs, 18-K-tile bf16 GEMM, +bias, store
"""

import numpy as np

import concourse.bass as bass
import concourse.bacc as bacc
import concourse.mybir as mybir
import concourse.tile as tile
from concourse.bass_utils import run_bass_kernel_spmd

B, CIN, COUT, H, W = 8, 256, 256, 56, 56
KH = KW = 3
NTAP = 9
HW = H * W  # 3136
NCORES = 8

FP = mybir.dt.float32
BF = mybir.dt.bfloat16
I16 = mybir.dt.int16
AL = mybir.AluOpType
AF = mybir.ActivationFunctionType

# position-dimension splits: (start, valid, nchunks_of_128)
SPLITS = [(0, 1152, 9), (1152, 1152, 9), (2304, 832, 7)]
GT_COLS = sum(2 * NTAP * nch for (_, _, nch) in SPLITS)  # 450


def _gt_colbase(sp):
    return sum(2 * NTAP * SPLITS[i][2] for i in range(sp))


def _emit_prep(nc, tc, x_d, w_d, id_sb, pstp, dramp, wpp):
    """Build xT (DRAM, bf16) and the 18 transposed weight tiles."""
    xT_d = dramp.tile([HW, CIN], BF, tag="xT")
    Wp = []
    with tc.tile_pool(name="prep", bufs=1) as prepp:
        xbs = []
        for cb in range(2):
            xf = prepp.tile([128, HW], FP, tag=f"xf{cb}")
            nc.sync.dma_start(xf[:], x_d.ap()[cb * 128:(cb + 1) * 128, :])
            xb = prepp.tile([128, HW], BF, tag=f"xb{cb}")
            nc.scalar.copy(xb[:], xf[:])
            xbs.append(xb)
        for j in range(25):
            wdt = 128 if j < 24 else 64
            stg = prepp.tile([128, CIN], BF, tag="xstg")
            for cb in range(2):
                ps = pstp.tile([128, 256], BF, tag="ps")
                nc.tensor.transpose(
                    ps[:wdt, :128], xbs[cb][:, j * 128:j * 128 + wdt], id_sb[:]
                )
                nc.vector.tensor_copy(stg[:wdt, cb * 128:(cb + 1) * 128], ps[:wdt, :128])
            nc.sync.dma_start(xT_d[j * 128:j * 128 + wdt, :], stg[:wdt, :])

        wbs = []
        for ob in range(2):
            wf = prepp.tile([128, CIN * NTAP], FP, tag=f"wf{ob}")
            nc.sync.dma_start(wf[:], w_d.ap()[ob * 128:(ob + 1) * 128, :])
            wb = prepp.tile([128, CIN * NTAP], BF, tag=f"wb{ob}")
            nc.scalar.copy(wb[:], wf[:])
            wbs.append(wb)
        for t in range(NTAP):
            for cb in range(2):
                wt = wpp.tile([128, COUT], BF, tag=f"wp{t}_{cb}")
                for ob in range(2):
                    ps = pstp.tile([128, 256], BF, tag="ps")
                    src = wbs[ob][:].rearrange("p (c k) -> p c k", k=NTAP)[
                        :, cb * 128:(cb + 1) * 128, t
                    ]
                    nc.tensor.transpose(ps[:, :128], src, id_sb[:])
                    nc.vector.tensor_copy(wt[:, ob * 128:(ob + 1) * 128], ps[:, :128])
                Wp.append(wt)
    return xT_d, Wp


def _emit_coords(nc, tc, sp, off_d, gt, idf32, coordp, livep, dramp, pstp):
    """Bilinear weights (4x [128, 9*nch] f32) + 16-wrapped int16 gather indices."""
    g0, valid, nch = SPLITS[sp]
    n9 = NTAP * nch
    full_ch = valid // 128
    rem = valid % 128

    # load offsets/mask row-major [27, valid], then PE-transpose each 128-col
    # chunk to the position-major layout offs[p, s*27 + r]
    offn = coordp.tile([32, nch * 128], FP, tag="offn")
    nc.sync.dma_start(offn[0:27, 0:valid], off_d.ap()[:, g0:g0 + valid])
    offs = coordp.tile([128, 27 * nch], FP, tag="offs")
    o3 = offs[:].rearrange("p (s r) -> p r s", r=27)
    if rem:
        nc.vector.memset(offs[rem:128, full_ch * 27:(full_ch + 1) * 27], 0.0)
    for s in range(nch):
        cw = 128 if s < full_ch else rem
        if cw == 0:
            break
        ps = pstp.tile([128, 256], FP, tag="ps")
        nc.tensor.transpose(
            ps[:cw, 0:27], offn[0:27, s * 128:s * 128 + cw], idf32[0:27, 0:27]
        )
        nc.vector.tensor_copy(offs[:cw, s * 27:(s + 1) * 27], ps[:cw, 0:27])
    di = o3[:, 0:18:2, :]
    dj = o3[:, 1:18:2, :]
    mm = o3[:, 18:27, :]
    cb_ = _gt_colbase(sp)
    gtr = gt[:, cb_:cb_ + n9].rearrange("p (t s) -> p t s", s=nch)
    gtc = gt[:, cb_ + n9:cb_ + 2 * n9].rearrange("p (t s) -> p t s", s=nch)

    def T9(tag):
        t_ = coordp.tile([128, n9], FP, tag=tag)
        return t_[:].rearrange("p (t s) -> p t s", s=nch)

    def emit_floor_frac(cc, lo, fr):
        """lo = floor(cc), fr = cc - lo, robust to the f32->int rounding mode."""
        cvi = coordp.tile([128, n9], mybir.dt.int32, tag="cvi")
        nc.vector.tensor_copy(cvi[:].rearrange("p (t s) -> p t s", s=nch), cc)
        cvf = T9("cvf")
        nc.vector.tensor_copy(cvf, cvi[:].rearrange("p (t s) -> p t s", s=nch))
        cmp = T9("cmpf")
        nc.vector.tensor_tensor(cmp, cvf, cc, op=AL.is_gt)
        nc.vector.tensor_sub(lo, cvf, cmp)
        nc.vector.tensor_sub(fr, cc, lo)

    ci = T9("ci")
    nc.vector.tensor_add(ci, di, gtr)
    fi = T9("fi")
    li = T9("li")
    emit_floor_frac(ci, li, fi)
    cj = T9("cj")
    nc.vector.tensor_add(cj, dj, gtc)
    fj = T9("fj")
    lj = T9("lj")
    emit_floor_frac(cj, lj, fj)

    lic = T9("lic")
    nc.vector.tensor_scalar(lic, li, 0.0, 55.0, op0=AL.max, op1=AL.min)
    ljc = T9("ljc")
    nc.vector.tensor_scalar(ljc, lj, 0.0, 55.0, op0=AL.max, op1=AL.min)
    lip = T9("lip")
    nc.vector.tensor_scalar(lip, li, 1.0, None, op0=AL.add)
    ljp = T9("ljp")
    nc.vector.tensor_scalar(ljp, lj, 1.0, None, op0=AL.add)
    ric = T9("ric")
    nc.vector.tensor_scalar(ric, lip, 0.0, 55.0, op0=AL.max, op1=AL.min)
    rjc = T9("rjc")
    nc.vector.tensor_scalar(rjc, ljp, 0.0, 55.0, op0=AL.max, op1=AL.min)

    vi0 = T9("vi0")
    nc.vector.tensor_tensor(vi0, lic, li, op=AL.is_equal)
    vi1 = T9("vi1")
    nc.vector.tensor_tensor(vi1, ric, lip, op=AL.is_equal)
    vj0 = T9("vj0")
    nc.vector.tensor_tensor(vj0, ljc, lj, op=AL.is_equal)
    vj1 = T9("vj1")
    nc.vector.tensor_tensor(vj1, rjc, ljp, op=AL.is_equal)

    a0 = T9("a0")
    nc.vector.tensor_scalar(a0, fi, -1.0, 1.0, op0=AL.mult, op1=AL.add)
    nc.vector.tensor_mul(a0, a0, vi0)
    nc.vector.tensor_mul(a0, a0, mm)
    a1 = T9("a1")
    nc.vector.tensor_mul(a1, fi, vi1)
    nc.vector.tensor_mul(a1, a1, mm)
    b0 = T9("b0")
    nc.vector.tensor_scalar(b0, fj, -1.0, 1.0, op0=AL.mult, op1=AL.add)
    nc.vector.tensor_mul(b0, b0, vj0)
    b1 = T9("b1")
    nc.vector.tensor_mul(b1, fj, vj1)

    wq = []
    for q, (aa, bb) in enumerate(((a0, b0), (a0, b1), (a1, b0), (a1, b1))):
        wt_ = livep.tile([128, n9], FP, tag=f"wq{q}")
        nc.vector.tensor_mul(wt_[:].rearrange("p (t s) -> p t s", s=nch), aa, bb)
        if rem:
            nc.vector.memset(
                wt_[:].rearrange("p (t s) -> p t s", s=nch)[
                    rem:128, :, full_ch:full_ch + 1
                ],
                0.0,
            )
        wq.append(wt_)

    # gather indices, f32 -> int16; idxi col layout = (t*4+q)*nch + s
    idxf = coordp.tile([128, 4 * n9], FP, tag="idxf")
    if4 = idxf[:].rearrange("p (t q s) -> p q t s", q=4, s=nch)
    nc.vector.scalar_tensor_tensor(if4[:, 0], lic, 56.0, ljc, op0=AL.mult, op1=AL.add)
    nc.vector.scalar_tensor_tensor(if4[:, 1], lic, 56.0, rjc, op0=AL.mult, op1=AL.add)
    nc.vector.scalar_tensor_tensor(if4[:, 2], ric, 56.0, ljc, op0=AL.mult, op1=AL.add)
    nc.vector.scalar_tensor_tensor(if4[:, 3], ric, 56.0, rjc, op0=AL.mult, op1=AL.add)
    # int32 row indices in plain [128, (t,q,s)] layout for indirect_dma_start
    idxi = livep.tile([128, 4 * n9], mybir.dt.int32, tag="idxi")
    nc.vector.tensor_copy(idxi[:], idxf[:])
    return wq, idxi


def _emit_split(nc, tc, sp, xT_d, Wp, btiles, id_sb, out_d, wq, idx16,
                gbp, rhsp, cmbp, ostp, pstp, pmmp):
    g0, valid, nch = SPLITS[sp]
    rhs_t = []
    for t in range(NTAP):
        G = gbp.tile([128, 4 * nch * 256], BF, tag="G")
        G3 = G[:].rearrange("p (c e) -> p c e", e=256)
        # verified HW form: dest [128, E] with one row index per partition
        for c in range(4 * nch):
            col = t * 4 * nch + c
            nc.gpsimd.indirect_dma_start(
                G3[:, c, :],
                None,
                xT_d[:],
                bass.IndirectOffsetOnAxis(ap=idx16[:, col:col + 1], axis=0),
            )
        rt = rhsp.tile([128, 2 * nch * 128], BF, tag=f"rhs{t}")
        rt3 = rt[:].rearrange("p (c s e) -> p c s e", c=2, s=nch)
        for s in range(nch):
            tmp0 = cmbp.tile([128, 256], BF, tag="tmp0")
            tmp1 = cmbp.tile([128, 256], BF, tag="tmp1")
            tmp2 = cmbp.tile([128, 256], BF, tag="tmp2")
            vcb = cmbp.tile([128, 256], BF, tag="vcb")

            def wsl(q):
                return wq[q][:, t * nch + s:t * nch + s + 1]

            nc.scalar.activation(tmp0[:], G3[:, 0 * nch + s, :], AF.Copy, scale=wsl(0))
            nc.vector.scalar_tensor_tensor(
                tmp1[:], G3[:, 1 * nch + s, :], wsl(1), tmp0[:],
                op0=AL.mult, op1=AL.add,
            )
            nc.vector.scalar_tensor_tensor(
                tmp2[:], G3[:, 2 * nch + s, :], wsl(2), tmp1[:],
                op0=AL.mult, op1=AL.add,
            )
            nc.vector.scalar_tensor_tensor(
                vcb[:], G3[:, 3 * nch + s, :], wsl(3), tmp2[:],
                op0=AL.mult, op1=AL.add,
            )
            ps = pstp.tile([128, 256], BF, tag="ps")
            for cc in range(2):
                nc.tensor.transpose(
                    ps[:, cc * 128:(cc + 1) * 128],
                    vcb[:, cc * 128:(cc + 1) * 128],
                    id_sb[:],
                )
            nc.vector.tensor_copy(
                rt3[:, :, s, :], ps[:].rearrange("p (c e) -> p c e", c=2)
            )
        rhs_t.append(rt)

    col0 = 0
    while col0 < valid:
        nb = min(512, nch * 128 - col0)
        vb = min(512, valid - col0)
        for ob in range(2):
            pm = pmmp.tile([128, 512], FP, tag="pm")
            kb = 0
            for t in range(NTAP):
                for cc in range(2):
                    nc.tensor.matmul(
                        pm[:, :nb],
                        Wp[t * 2 + cc][:, ob * 128:(ob + 1) * 128],
                        rhs_t[t][:, cc * nch * 128 + col0:cc * nch * 128 + col0 + nb],
                        start=(kb == 0),
                        stop=(kb == 17),
                    )
                    kb += 1
            ot = ostp.tile([128, 512], FP, tag="ot")
            nc.vector.tensor_scalar(
                ot[:, :vb], pm[:, :vb], btiles[ob][:, 0:1], None, op0=AL.add
            )
            nc.sync.dma_start(
                out_d.ap()[ob * 128:(ob + 1) * 128, g0 + col0:g0 + col0 + vb],
                ot[:, :vb],
            )
        col0 += 512


def _emit_kernel(nc, x_d, off_d, w_d, b_d, gt_d, id_d, out_d):
    with tile.TileContext(nc) as tc:
        with (
            tc.tile_pool(name="const", bufs=1) as constp,
            tc.tile_pool(name="wp", bufs=1) as wpp,
            tc.tile_pool(name="pst", bufs=3, space="PSUM") as pstp,
            tc.tile_pool(name="pmm", bufs=4, space="PSUM") as pmmp,
            tc.tile_pool(name="dram", bufs=1, space="DRAM") as dramp,
        ):
            id_sb = constp.tile([128, 128], BF)
            nc.sync.dma_start(id_sb[:], id_d.ap())
            idf32 = constp.tile([128, 128], FP)
            nc.vector.tensor_copy(idf32[:], id_sb[:])
            gt = constp.tile([128, GT_COLS], FP)
            nc.sync.dma_start(gt[:], gt_d.ap())
            btiles = []
            for ob in range(2):
                bt = constp.tile([128, 1], FP, tag=f"bias{ob}")
                nc.sync.dma_start(bt[:], b_d.ap()[ob * 128:(ob + 1) * 128, :])
                btiles.append(bt)

            xT_d, Wp = _emit_prep(nc, tc, x_d, w_d, id_sb, pstp, dramp, wpp)

            with (
                tc.tile_pool(name="gbuf", bufs=2) as gbp,
                tc.tile_pool(name="rhs", bufs=2) as rhsp,
                tc.tile_pool(name="coord", bufs=1) as coordp,
                tc.tile_pool(name="live", bufs=2) as livep,
                tc.tile_pool(name="cmb", bufs=4) as cmbp,
                tc.tile_pool(name="ost", bufs=3) as ostp,
            ):
                for sp in range(len(SPLITS)):
                    wq, idx16 = _emit_coords(nc, tc, sp, off_d, gt, idf32,
                                             coordp, livep, dramp, pstp)
                    _emit_split(nc, tc, sp, xT_d, Wp, btiles, id_sb, out_d,
                                wq, idx16, gbp, rhsp, cmbp, ostp, pstp, pmmp)


def build_nc():
    nc = bacc.Bacc(
        "TRN2",
        target_bir_lowering=False,
        debug=False,
        num_devices=NCORES,
    )
    x_d = nc.dram_tensor("x", [CIN, HW], FP, kind="ExternalInput")
    off_d = nc.dram_tensor("offm", [27, HW], FP, kind="ExternalInput")
    w_d = nc.dram_tensor("w", [COUT, CIN * NTAP], FP, kind="ExternalInput")
    b_d = nc.dram_tensor("bias", [COUT, 1], FP, kind="ExternalInput")
    gt_d = nc.dram_tensor("gtab", [128, GT_COLS], FP, kind="ExternalInput")
    id_d = nc.dram_tensor("ident", [128, 128], BF, kind="ExternalInput")
    out_d = nc.dram_tensor("out", [COUT, HW], FP, kind="ExternalOutput")
    _emit_kernel(nc, x_d, off_d, w_d, b_d, gt_d, id_d, out_d)
    nc.compile()
    return nc


def make_gtab():
    gtab = np.zeros((128, GT_COLS), np.float32)
    p = np.arange(128)
    for sp, (g0, valid, nch) in enumerate(SPLITS):
        cb_ = _gt_colbase(sp)
        for ax in range(2):
            for t in range(NTAP):
                for s in range(nch):
                    g = g0 + s * 128 + p
                    ok = g < g0 + valid
                    gc = np.where(ok, g, 0)
                    if ax == 0:
                        val = gc // 56 + (t // 3 - 1)
                    else:
                        val = gc % 56 + (t % 3 - 1)
                    gtab[:, cb_ + (ax * NTAP + t) * nch + s] = np.where(ok, val, 0.0)
    return gtab


_NC_CACHE = {}


def kernel(x, offset, mask, weight, bias):
    import ml_dtypes

    x = np.ascontiguousarray(np.asarray(x, np.float32))
    offset = np.ascontiguousarray(np.asarray(offset, np.float32))
    mask = np.ascontiguousarray(np.asarray(mask, np.float32))
    weight = np.ascontiguousarray(np.asarray(weight, np.float32))
    bias = np.ascontiguousarray(np.asarray(bias, np.float32))

    if "nc" not in _NC_CACHE:
        _NC_CACHE["nc"] = build_nc()
    nc = _NC_CACHE["nc"]

    gtab = make_gtab()
    ident = np.eye(128, dtype=np.float32).astype(ml_dtypes.bfloat16)
    wmat = np.ascontiguousarray(weight.reshape(COUT, CIN * NTAP))
    bcol = np.ascontiguousarray(bias.reshape(COUT, 1))

    in_maps = []
    for i in range(NCORES):
        offm = np.ascontiguousarray(
            np.concatenate(
                [offset[i].reshape(18, HW), mask[i].reshape(NTAP, HW)], axis=0
            )
        )
        in_maps.append(
            {
                "x": np.ascontiguousarray(x[i].reshape(CIN, HW)),
                "offm": offm,
                "w": wmat,
                "bias": bcol,
                "gtab": gtab,
                "ident": ident,
            }
        )

    res = run_bass_kernel_spmd(nc, in_maps, core_ids=list(range(NCORES)))
    out = np.stack([r["out"] for r in res.results], axis=0)
    return np.ascontiguousarray(out.reshape(B, COUT, H, W).astype(np.float32))



# revision 7
# speedup vs baseline: 6336.3163x; 6336.3163x over previous
"""Modulated deformable conv2d (DCNv2) on Trainium2, data-parallel over batch on 8 NeuronCores.

Per-core pipeline (one batch element per core):
  1. prep:  x [256,3136] f32 -> bf16 -> PE-transpose -> xT4 [3200,1024] bf16 in DRAM,
            where xT4[m] = [xT[m-57] | xT[m-56] | xT[m-1] | xT[m]] -- the four bilinear
            neighbors (lt, rt, lb, rb) of sample point r = m-57 are one contiguous 2KB
            block; out-of-image guard rows are zeroed.
            weight [256,2304] f32 -> bf16 -> PE-transpose -> W' tiles [128(k),256(o)]
  2. coords: offsets+mask -> fractional bilinear weights (position-major, per-partition
            scalars) + int16 row index m = 56*clip(li+1,0,56) + clip(lj+1,0,56), wrapped
            into dma_gather's [16, n/16] layout and replicated across partition groups
  3. gather: ONE dma_gather per (split, tap): 128*nch indices x 2KB -> G [128, nch, 1024]
            (batched SWDGE descriptor generation: ~1us fixed + 0.34ns/desc)
  4. combine: w_lt*v_lt + w_rt*v_rt + w_lb*v_lb + w_rb*v_rb  (1 ACT mul + 3 DVE fused ops)
  5. PE-transpose combined tiles into channel-major rhs, 18-K-tile bf16 GEMM, +bias, store
"""

import os

import numpy as np

import concourse.bass as bass
import concourse.bacc as bacc
import concourse.mybir as mybir
import concourse.tile as tile
from concourse.bass_utils import run_bass_kernel_spmd

# "gather" = one dma_gather per (split, tap); "ind4" = per-chunk indirect_dma_start
VARIANT = os.environ.get("DCN_VARIANT", "gather")

B, CIN, COUT, H, W = 8, 256, 256, 56, 56
KH = KW = 3
NTAP = 9
HW = H * W  # 3136
NCORES = 8
XT4_ROWS = 3200  # gather indices go up to 3192

FP = mybir.dt.float32
BF = mybir.dt.bfloat16
I16 = mybir.dt.int16
AL = mybir.AluOpType
AF = mybir.ActivationFunctionType

# position-dimension splits: (start, valid, nchunks_of_128)
SPLITS = [(0, 1152, 9), (1152, 1152, 9), (2304, 832, 7)]
GT_COLS = sum(2 * NTAP * nch for (_, _, nch) in SPLITS)  # 450


def _gt_colbase(sp):
    return sum(2 * NTAP * SPLITS[i][2] for i in range(sp))


def _emit_prep(nc, tc, x_d, w_d, id_sb, pstp, dramp, wpp):
    """Build xT4 (DRAM, bf16, 4-neighbor packed) and the 18 transposed weight tiles."""
    xT4_d = dramp.tile([XT4_ROWS, 4 * CIN], BF, tag="xT4")
    # block k of xT4 holds xT[m - 57 + OFFK[k]]; chunk rows r land at m = r + 57 - OFFK[k]
    OFFK = (0, 1, 56, 57)
    Wp = []
    with tc.tile_pool(name="prep", bufs=1) as prepp:
        # zero the guard rows first; valid data overwrites below
        zt = prepp.tile([64, 4 * CIN], BF, tag="zt")
        nc.vector.memset(zt[:], 0.0)
        nc.sync.dma_start(xT4_d[0:57, :], zt[0:57, :])
        nc.sync.dma_start(xT4_d[3136:3200, :], zt[0:64, :])

        xbs = []
        for cb in range(2):
            xf = prepp.tile([128, HW], FP, tag=f"xf{cb}")
            nc.sync.dma_start(xf[:], x_d.ap()[cb * 128:(cb + 1) * 128, :])
            xb = prepp.tile([128, HW], BF, tag=f"xb{cb}")
            nc.scalar.copy(xb[:], xf[:])
            xbs.append(xb)
        # super-chunks: 6 of 4 chunks + 1 of 1 chunk (chunk 24 is 64 rows)
        for j0, njc in [(0, 4), (4, 4), (8, 4), (12, 4), (16, 4), (20, 4), (24, 1)]:
            wdt = 128 if j0 < 24 else 64
            stg = prepp.tile([128, njc * 256], BF, tag="xstg")
            for c in range(njc):
                j = j0 + c
                for cb in range(2):
                    ps = pstp.tile([128, 256], BF, tag="ps")
                    nc.tensor.transpose(
                        ps[:wdt, :128], xbs[cb][:, j * 128:j * 128 + wdt], id_sb[:]
                    )
                    nc.vector.tensor_copy(
                        stg[:wdt, c * 256 + cb * 128:c * 256 + (cb + 1) * 128],
                        ps[:wdt, :128],
                    )
            for k in range(4):
                m0 = j0 * 128 + 57 - OFFK[k]
                dst = xT4_d[m0:m0 + (njc - 1) * 128 + wdt, 256 * k:256 * (k + 1)]
                if njc > 1:
                    dst = dst.rearrange("(c p) e -> p c e", c=njc)
                    nc.sync.dma_start(
                        dst, stg[:].rearrange("p (c e) -> p c e", c=njc)
                    )
                else:
                    nc.sync.dma_start(dst, stg[:wdt, 0:256])

        wbs = []
        for ob in range(2):
            wf = prepp.tile([128, CIN * NTAP], FP, tag=f"wf{ob}")
            nc.sync.dma_start(wf[:], w_d.ap()[ob * 128:(ob + 1) * 128, :])
            wb = prepp.tile([128, CIN * NTAP], BF, tag=f"wb{ob}")
            nc.scalar.copy(wb[:], wf[:])
            wbs.append(wb)
        for t in range(NTAP):
            for cb in range(2):
                wt = wpp.tile([128, COUT], BF, tag=f"wp{t}_{cb}")
                for ob in range(2):
                    ps = pstp.tile([128, 256], BF, tag="ps")
                    src = wbs[ob][:].rearrange("p (c k) -> p c k", k=NTAP)[
                        :, cb * 128:(cb + 1) * 128, t
                    ]
                    nc.tensor.transpose(ps[:, :128], src, id_sb[:])
                    nc.vector.tensor_copy(wt[:, ob * 128:(ob + 1) * 128], ps[:, :128])
                Wp.append(wt)
    return xT4_d, Wp


def _emit_coords(nc, tc, sp, off_d, gt, idf32, coordp, livep, dramp, pstp):
    """Bilinear weights (4x [128, 9*nch] f32) + wrapped int16 gather indices."""
    g0, valid, nch = SPLITS[sp]
    n9 = NTAP * nch
    full_ch = valid // 128
    rem = valid % 128

    # load offsets/mask row-major [27, valid], then PE-transpose each 128-col
    # chunk to the position-major layout offs[p, s*27 + r]
    offn = coordp.tile([32, nch * 128], FP, tag="offn")
    nc.sync.dma_start(offn[0:27, 0:valid], off_d.ap()[:, g0:g0 + valid])
    offs = coordp.tile([128, 27 * nch], FP, tag="offs")
    o3 = offs[:].rearrange("p (s r) -> p r s", r=27)
    if rem:
        nc.vector.memset(offs[rem:128, full_ch * 27:(full_ch + 1) * 27], 0.0)
    for s in range(nch):
        cw = 128 if s < full_ch else rem
        if cw == 0:
            break
        ps = pstp.tile([128, 256], FP, tag="ps")
        nc.tensor.transpose(
            ps[:cw, 0:27], offn[0:27, s * 128:s * 128 + cw], idf32[0:27, 0:27]
        )
        nc.vector.tensor_copy(offs[:cw, s * 27:(s + 1) * 27], ps[:cw, 0:27])
    di = o3[:, 0:18:2, :]
    dj = o3[:, 1:18:2, :]
    mm = o3[:, 18:27, :]
    cb_ = _gt_colbase(sp)
    gtr = gt[:, cb_:cb_ + n9].rearrange("p (t s) -> p t s", s=nch)
    gtc = gt[:, cb_ + n9:cb_ + 2 * n9].rearrange("p (t s) -> p t s", s=nch)

    def T9(tag):
        t_ = coordp.tile([128, n9], FP, tag=tag)
        return t_[:].rearrange("p (t s) -> p t s", s=nch)

    def emit_floor_frac(cc, lo, fr):
        """lo = floor(cc), fr = cc - lo, robust to the f32->int rounding mode."""
        cvi = coordp.tile([128, n9], mybir.dt.int32, tag="cvi")
        nc.vector.tensor_copy(cvi[:].rearrange("p (t s) -> p t s", s=nch), cc)
        cvf = T9("cvf")
        nc.vector.tensor_copy(cvf, cvi[:].rearrange("p (t s) -> p t s", s=nch))
        cmp = T9("cmpf")
        nc.vector.tensor_tensor(cmp, cvf, cc, op=AL.is_gt)
        nc.vector.tensor_sub(lo, cvf, cmp)
        nc.vector.tensor_sub(fr, cc, lo)

    ci = T9("ci")
    nc.vector.tensor_add(ci, di, gtr)
    fi = T9("fi")
    li = T9("li")
    emit_floor_frac(ci, li, fi)
    cj = T9("cj")
    nc.vector.tensor_add(cj, dj, gtc)
    fj = T9("fj")
    lj = T9("lj")
    emit_floor_frac(cj, lj, fj)

    lic = T9("lic")
    nc.vector.tensor_scalar(lic, li, 0.0, 55.0, op0=AL.max, op1=AL.min)
    ljc = T9("ljc")
    nc.vector.tensor_scalar(ljc, lj, 0.0, 55.0, op0=AL.max, op1=AL.min)
    lip = T9("lip")
    nc.vector.tensor_scalar(lip, li, 1.0, None, op0=AL.add)
    ljp = T9("ljp")
    nc.vector.tensor_scalar(ljp, lj, 1.0, None, op0=AL.add)
    ric = T9("ric")
    nc.vector.tensor_scalar(ric, lip, 0.0, 55.0, op0=AL.max, op1=AL.min)
    rjc = T9("rjc")
    nc.vector.tensor_scalar(rjc, ljp, 0.0, 55.0, op0=AL.max, op1=AL.min)

    vi0 = T9("vi0")
    nc.vector.tensor_tensor(vi0, lic, li, op=AL.is_equal)
    vi1 = T9("vi1")
    nc.vector.tensor_tensor(vi1, ric, lip, op=AL.is_equal)
    vj0 = T9("vj0")
    nc.vector.tensor_tensor(vj0, ljc, lj, op=AL.is_equal)
    vj1 = T9("vj1")
    nc.vector.tensor_tensor(vj1, rjc, ljp, op=AL.is_equal)

    a0 = T9("a0")
    nc.vector.tensor_scalar(a0, fi, -1.0, 1.0, op0=AL.mult, op1=AL.add)
    nc.vector.tensor_mul(a0, a0, vi0)
    nc.vector.tensor_mul(a0, a0, mm)
    a1 = T9("a1")
    nc.vector.tensor_mul(a1, fi, vi1)
    nc.vector.tensor_mul(a1, a1, mm)
    b0 = T9("b0")
    nc.vector.tensor_scalar(b0, fj, -1.0, 1.0, op0=AL.mult, op1=AL.add)
    nc.vector.tensor_mul(b0, b0, vj0)
    b1 = T9("b1")
    nc.vector.tensor_mul(b1, fj, vj1)

    wq = []
    for q, (aa, bb) in enumerate(((a0, b0), (a0, b1), (a1, b0), (a1, b1))):
        wt_ = livep.tile([128, n9], FP, tag=f"wq{q}")
        nc.vector.tensor_mul(wt_[:].rearrange("p (t s) -> p t s", s=nch), aa, bb)
        if rem:
            nc.vector.memset(
                wt_[:].rearrange("p (t s) -> p t s", s=nch)[
                    rem:128, :, full_ch:full_ch + 1
                ],
                0.0,
            )
        wq.append(wt_)

    # gather row index m = 56*clip(li+1, 0, 56) + clip(lj+1, 0, 56); col layout (t s)
    ri1 = T9("ri1")
    nc.vector.tensor_scalar(ri1, lip, 0.0, 56.0, op0=AL.max, op1=AL.min)
    rj1 = T9("rj1")
    nc.vector.tensor_scalar(rj1, ljp, 0.0, 56.0, op0=AL.max, op1=AL.min)
    midf = coordp.tile([128, n9], FP, tag="midf")
    nc.vector.scalar_tensor_tensor(
        midf[:].rearrange("p (t s) -> p t s", s=nch), ri1, 56.0, rj1,
        op0=AL.mult, op1=AL.add,
    )
    if VARIANT == "ind4":
        idxi = livep.tile([128, n9], mybir.dt.int32, tag="idxi")
        nc.vector.tensor_copy(idxi[:], midf[:])
        return wq, idxi

    mi16 = coordp.tile([128, n9], I16, tag="mi16")
    nc.vector.tensor_copy(mi16[:], midf[:])

    # rewrap to dma_gather index layout: idx i=(s*128+p) -> [p%16, (t*nch+s)*8 + p//16],
    # then replicate the 16-partition block across all 128 partitions.
    scr = dramp.tile([128, n9], I16, tag=f"iscr{sp % 2}")
    nc.sync.dma_start(scr[:, :], mi16[:])
    idxw = livep.tile([128, 8 * n9], I16, tag="idxw")
    # idxw[l, (c g)] = scr[(g l), c]: flat src stride l=n9, c=1, g=16*n9
    nc.sync.dma_start(
        idxw[0:16, :].rearrange("l (c g) -> l c g", g=8),
        scr[:, :].rearrange("(g l) c -> l c g", g=8),
    )
    nc.sync.dma_start(idxw[16:32, :], idxw[0:16, :])
    nc.sync.dma_start(idxw[32:64, :], idxw[0:32, :])
    nc.sync.dma_start(idxw[64:128, :], idxw[0:64, :])
    return wq, idxw


def _emit_split(nc, tc, sp, xT4_d, Wp, btiles, id_sb, out_d, wq, idxw,
                gbp, rhsp, cmbp, ostp, pstp, pmmp):
    g0, valid, nch = SPLITS[sp]
    rhs_t = []
    for t in range(NTAP):
        G = gbp.tile([128, nch * 1024], BF, tag="G")
        G3 = G[:].rearrange("p (s e) -> p s e", e=1024)
        if VARIANT == "ind4":
            for s in range(nch):
                col = t * nch + s
                nc.gpsimd.indirect_dma_start(
                    G3[:, s, :],
                    None,
                    xT4_d[:],
                    bass.IndirectOffsetOnAxis(ap=idxw[:, col:col + 1], axis=0),
                )
        else:
            nc.gpsimd.dma_gather(
                G3,
                xT4_d[:, :],
                idxw[:, t * 8 * nch:(t + 1) * 8 * nch],
                num_idxs=128 * nch,
                num_idxs_reg=128 * nch,
                elem_size=1024,
            )
        rt = rhsp.tile([128, 2 * nch * 128], BF, tag=f"rhs{t}")
        rt3 = rt[:].rearrange("p (c s e) -> p c s e", c=2, s=nch)
        for s in range(nch):
            tmp0 = cmbp.tile([128, 256], BF, tag="tmp0")
            tmp1 = cmbp.tile([128, 256], BF, tag="tmp1")
            tmp2 = cmbp.tile([128, 256], BF, tag="tmp2")
            vcb = cmbp.tile([128, 256], BF, tag="vcb")

            def wsl(q):
                return wq[q][:, t * nch + s:t * nch + s + 1]

            def gsl(q):
                return G3[:, s, q * 256:(q + 1) * 256]

            nc.scalar.activation(tmp0[:], gsl(0), AF.Copy, scale=wsl(0))
            nc.vector.scalar_tensor_tensor(
                tmp1[:], gsl(1), wsl(1), tmp0[:], op0=AL.mult, op1=AL.add
            )
            nc.vector.scalar_tensor_tensor(
                tmp2[:], gsl(2), wsl(2), tmp1[:], op0=AL.mult, op1=AL.add
            )
            nc.vector.scalar_tensor_tensor(
                vcb[:], gsl(3), wsl(3), tmp2[:], op0=AL.mult, op1=AL.add
            )
            ps = pstp.tile([128, 256], BF, tag="ps")
            for cc in range(2):
                nc.tensor.transpose(
                    ps[:, cc * 128:(cc + 1) * 128],
                    vcb[:, cc * 128:(cc + 1) * 128],
                    id_sb[:],
                )
            nc.vector.tensor_copy(
                rt3[:, :, s, :], ps[:].rearrange("p (c e) -> p c e", c=2)
            )
        rhs_t.append(rt)

    col0 = 0
    while col0 < valid:
        nb = min(512, nch * 128 - col0)
        vb = min(512, valid - col0)
        for ob in range(2):
            pm = pmmp.tile([128, 512], FP, tag="pm")
            kb = 0
            for t in range(NTAP):
                for cc in range(2):
                    nc.tensor.matmul(
                        pm[:, :nb],
                        Wp[t * 2 + cc][:, ob * 128:(ob + 1) * 128],
                        rhs_t[t][:, cc * nch * 128 + col0:cc * nch * 128 + col0 + nb],
                        start=(kb == 0),
                        stop=(kb == 17),
                    )
                    kb += 1
            ot = ostp.tile([128, 512], FP, tag="ot")
            nc.vector.tensor_scalar(
                ot[:, :vb], pm[:, :vb], btiles[ob][:, 0:1], None, op0=AL.add
            )
            nc.sync.dma_start(
                out_d.ap()[ob * 128:(ob + 1) * 128, g0 + col0:g0 + col0 + vb],
                ot[:, :vb],
            )
        col0 += 512


def _emit_kernel(nc, x_d, off_d, w_d, b_d, gt_d, id_d, out_d):
    with tile.TileContext(nc) as tc:
        with (
            tc.tile_pool(name="const", bufs=1) as constp,
            tc.tile_pool(name="wp", bufs=1) as wpp,
            tc.tile_pool(name="pst", bufs=3, space="PSUM") as pstp,
            tc.tile_pool(name="pmm", bufs=4, space="PSUM") as pmmp,
            tc.tile_pool(name="dram", bufs=1, space="DRAM") as dramp,
        ):
            id_sb = constp.tile([128, 128], BF)
            nc.sync.dma_start(id_sb[:], id_d.ap())
            idf32 = constp.tile([128, 128], FP)
            nc.vector.tensor_copy(idf32[:], id_sb[:])
            gt = constp.tile([128, GT_COLS], FP)
            nc.sync.dma_start(gt[:], gt_d.ap())
            btiles = []
            for ob in range(2):
                bt = constp.tile([128, 1], FP, tag=f"bias{ob}")
                nc.sync.dma_start(bt[:], b_d.ap()[ob * 128:(ob + 1) * 128, :])
                btiles.append(bt)

            xT4_d, Wp = _emit_prep(nc, tc, x_d, w_d, id_sb, pstp, dramp, wpp)

            with (
                tc.tile_pool(name="gbuf", bufs=2) as gbp,
                tc.tile_pool(name="rhs", bufs=2) as rhsp,
                tc.tile_pool(name="coord", bufs=1) as coordp,
                tc.tile_pool(name="live", bufs=2) as livep,
                tc.tile_pool(name="cmb", bufs=4) as cmbp,
                tc.tile_pool(name="ost", bufs=3) as ostp,
            ):
                for sp in range(len(SPLITS)):
                    wq, idxw = _emit_coords(nc, tc, sp, off_d, gt, idf32,
                                            coordp, livep, dramp, pstp)
                    _emit_split(nc, tc, sp, xT4_d, Wp, btiles, id_sb, out_d,
                                wq, idxw, gbp, rhsp, cmbp, ostp, pstp, pmmp)


def build_nc():
    nc = bacc.Bacc(
        "TRN2",
        target_bir_lowering=False,
        debug=False,
        num_devices=NCORES,
    )
    x_d = nc.dram_tensor("x", [CIN, HW], FP, kind="ExternalInput")
    off_d = nc.dram_tensor("offm", [27, HW], FP, kind="ExternalInput")
    w_d = nc.dram_tensor("w", [COUT, CIN * NTAP], FP, kind="ExternalInput")
    b_d = nc.dram_tensor("bias", [COUT, 1], FP, kind="ExternalInput")
    gt_d = nc.dram_tensor("gtab", [128, GT_COLS], FP, kind="ExternalInput")
    id_d = nc.dram_tensor("ident", [128, 128], BF, kind="ExternalInput")
    out_d = nc.dram_tensor("out", [COUT, HW], FP, kind="ExternalOutput")
    _emit_kernel(nc, x_d, off_d, w_d, b_d, gt_d, id_d, out_d)
    nc.compile()
    return nc


def make_gtab():
    gtab = np.zeros((128, GT_COLS), np.float32)
    p = np.arange(128)
    for sp, (g0, valid, nch) in enumerate(SPLITS):
        cb_ = _gt_colbase(sp)
        for ax in range(2):
            for t in range(NTAP):
                for s in range(nch):
                    g = g0 + s * 128 + p
                    ok = g < g0 + valid
                    gc = np.where(ok, g, 0)
                    if ax == 0:
                        val = gc // 56 + (t // 3 - 1)
                    else:
                        val = gc % 56 + (t % 3 - 1)
                    gtab[:, cb_ + (ax * NTAP + t) * nch + s] = np.where(ok, val, 0.0)
    return gtab


_NC_CACHE = {}


def kernel(x, offset, mask, weight, bias):
    import ml_dtypes

    x = np.ascontiguousarray(np.asarray(x, np.float32))
    offset = np.ascontiguousarray(np.asarray(offset, np.float32))
    mask = np.ascontiguousarray(np.asarray(mask, np.float32))
    weight = np.ascontiguousarray(np.asarray(weight, np.float32))
    bias = np.ascontiguousarray(np.asarray(bias, np.float32))

    if "nc" not in _NC_CACHE:
        _NC_CACHE["nc"] = build_nc()
    nc = _NC_CACHE["nc"]

    gtab = make_gtab()
    ident = np.eye(128, dtype=np.float32).astype(ml_dtypes.bfloat16)
    wmat = np.ascontiguousarray(weight.reshape(COUT, CIN * NTAP))
    bcol = np.ascontiguousarray(bias.reshape(COUT, 1))

    in_maps = []
    for i in range(NCORES):
        offm = np.ascontiguousarray(
            np.concatenate(
                [offset[i].reshape(18, HW), mask[i].reshape(NTAP, HW)], axis=0
            )
        )
        in_maps.append(
            {
                "x": np.ascontiguousarray(x[i].reshape(CIN, HW)),
                "offm": offm,
                "w": wmat,
                "bias": bcol,
                "gtab": gtab,
                "ident": ident,
            }
        )

    res = run_bass_kernel_spmd(nc, in_maps, core_ids=list(range(NCORES)))
    out = np.stack([r["out"] for r in res.results], axis=0)
    return np.ascontiguousarray(out.reshape(B, COUT, H, W).astype(np.float32))
